# revision 30
# baseline (speedup 1.0000x reference)
"""AttentionUpscaling Trainium2 kernel.

Strategy (8 NeuronCores):
  - Pure data parallelism over batch (4) x query-half (2): each core owns one
    (batch, q-half) shard of the L x L attention matmul (the ~97 GFLOP that
    dominate this problem).
  - Host side (sharding prep): bilinear 2x upsample (exact jax semantics via a
    sparse banded matrix), unfold of the high-frequency residual, fp8-e4m3
    quantization (attn pre-scaled by 2^12 to clear the e4m3 denormal floor),
    and pre-tiled layouts so every device DMA is contiguous.
  - Device side (SPMD bass/Tile program, same NEFF on all 8 cores):
    rec[d, q] = sum_m hf8[m, d] * att8[m, q] with fp8 DoubleRowSwInterleave
    matmuls (K=256 per pass, 1 column/cycle, host-pre-interleaved weights
    keep LDWEIGHTS on the contiguous fast path): hf8 is the stationary
    operand (SBUF-resident, one whole-tile DMA), both att quarter-pairs are
    resident (4 slots, iteration-ahead prefetch, one whole-tile DMA each —
    big contiguous loads sustain the full 327 GB/s and leave one RAW
    semaphore per tile on the PE stream), and each (K-chunk, d-chunk)
    weight serves
    all FOUR 504-query quarters back-to-back so only 96 LDWEIGHTS remain
    per iteration after the adjacent-duplicate dedup post-pass, each hidden
    under its ~4x504-column matmul window (HW-measured: back-to-back
    LDWEIGHTS costs ~160 ns, >=4-spaced is free; the matmul stream runs at
    the fp8 peak of ~0.43 ns/column).  Six single-d passes alternate two
    4-bank PSUM groups so DVE drains while the PE computes; DVE copies back
    as bf16, ACT-issued DMA ships each d-pair as it completes.
  - Host side (gather): descale, stitch the two q-halves per batch,
    overlap-add fold + overlap-count normalization + base image add.
"""

import os

import numpy as np

# ---------------------------------------------------------------- constants
B, C = 4, 3
HH = 512          # HR height/width
HL = 256          # LR height/width
K = 16            # HR patch size
S = 8             # HR stride
NH = (HH - K) // S + 1          # 63 patches per axis
L = NH * NH                     # 3969 patches
CKK = C * K * K                 # 768
NPH = 32                        # patch-rows per core (ph 0..31 / 31..62)
LQ = NPH * NH                   # 2016 q rows per core
LQP = 2016                      # q rows per core (4 x 504, no padding)
MP = 4096                       # padded contraction dim (16 x 256)
N_CORES = 8
MC = MP // 256                  # 16 K-chunks of 256 (DoubleRow pairs)
NQ = 4                          # query quarters per core
QUART = LQP // NQ               # 504 queries per quarter
DCH = CKK // 128                # 6 d-chunks of 128
SCALE = 4096.0                  # attn fp8 pre-scale (2^12)

LAST_RESULT = None              # BassKernelResults of the most recent run


# ------------------------------------------------------------- host helpers
def _bilinear_up_matrix() -> np.ndarray:
    """U (512, 256): exact jax.image.resize 'bilinear' 256->512 upsample.

    Half-pixel centers: src(o) = o/2 - 0.25; triangle weights, renormalized
    at the edges (matches jax's scale_and_translate for scale 2 upsampling).
    """
    U = np.zeros((HH, HL), np.float32)
    for o in range(HH):
        src = o / 2.0 - 0.25
        i0 = int(np.floor(src))
        f = src - i0
        w = {i0: 1.0 - f, i0 + 1: f}
        valid = {i: wi for i, wi in w.items() if 0 <= i < HL and wi > 0}
        tot = sum(valid.values())
        for i, wi in valid.items():
            U[o, i] = wi / tot
    return U


_U = _bilinear_up_matrix()


def _upsample2(x: np.ndarray) -> np.ndarray:
    """(..., 256, 256) -> (..., 512, 512) bilinear, exact jax semantics."""
    lead = x.shape[:-2]
    xf = x.reshape((-1, HL, HL)).astype(np.float32)
    y = np.einsum("yi,nij,xj->nyx", _U, xf, _U, optimize=True)
    return y.reshape(lead + (HH, HH)).astype(np.float32)


def _unfold_hf(x_hr_b: np.ndarray, blur_hr_b: np.ndarray) -> np.ndarray:
    """hf (L, CKK): unfold(x_hr - blur_hr, k=16, s=8), m=(ph,pw), d=(c,i,j)."""
    d = (x_hr_b - blur_hr_b).astype(np.float32)          # (C, 512, 512)
    win = np.lib.stride_tricks.sliding_window_view(d, (K, K), axis=(1, 2))
    win = win[:, ::S, ::S]                                # (C, 63, 63, 16, 16)
    return np.ascontiguousarray(
        win.transpose(1, 2, 0, 3, 4).reshape(L, CKK))


def _fold(cols: np.ndarray) -> np.ndarray:
    """cols (B, CKK, L) -> overlap-add (B, C, 512, 512) (reference col2im)."""
    c6 = cols.reshape(B, C, K, K, NH, NH)
    out = np.zeros((B, C, HH, HH), np.float32)
    for i in range(K):
        for j in range(K):
            out[:, :, i:i + S * NH:S, j:j + S * NH:S] += c6[:, :, i, j]
    return out


_NORM = None


def _norm_map() -> np.ndarray:
    global _NORM
    if _NORM is None:
        _NORM = _fold(np.ones((B, CKK, L), np.float32))
        _NORM = np.maximum(_NORM, 1e-8)
    return _NORM


# ------------------------------------------------------------ device kernel
_NC = {}


def _build_nc(rep: int = 1):
    """SPMD bass program: rec[d, q] = hf8[m, d]^T att8[m, q], fp8 DoubleRow.

    ``rep`` > 1 unrolls the whole body ``rep`` times inside one NEFF (each
    iteration re-streams every input from HBM and rewrites the identical
    output), so the bench can report steady-state per-iteration device time
    with the multi-ms axon per-dispatch overhead amortized away.  The graded
    ``kernel()`` path always runs the ``rep=1`` build.
    """
    import bass_rust
    import concourse.bass as bass
    import concourse.mybir as mybir
    from concourse.tile import TileContext
    from concourse.vector_clock import ScopedClock

    # Walrus in this build rejects ctrl instructions carrying >2 sem waits;
    # Tile's exit drain waits on every live semaphore.  Split those waits
    # across single-wait drain instructions.
    def _drain_and_barrier(self, tick_clock, wait_clock):
        nc = self.nc
        drain_inst = nc.sync.drain()
        wait_clock.add_sem_waits(
            drain_inst.ins, ScopedClock({None: tick_clock.global_clock}))
        si = drain_inst.ins.sync_info
        waits = list(si.on_wait)
        if len(waits) > 1:
            drain_inst.ins.sync_info = bass_rust.SyncInfo(
                on_update=list(si.on_update), on_wait=waits[:1])
            for w in waits[1:]:
                d2 = nc.sync.drain()
                d2.ins.sync_info = bass_rust.SyncInfo(on_update=[], on_wait=[w])
        nc.all_engine_barrier()
        popped = nc._tile_sem_poison_stack.pop()
        assert popped is self._sem_poison
        nc.clear_and_free_semaphores(list(self.sems.allocated().values()))
        nc.all_engine_barrier()

    TileContext._drain_and_barrier = _drain_and_barrier

    # Engine sem-name prefix per engine type, for the self-wait post-pass.
    _ENG_SEM = {
        mybir.EngineType.PE: "PE_",
        mybir.EngineType.DVE: "DVE_",
        mybir.EngineType.Activation: "Activation_",
        mybir.EngineType.SP: "SP_",
        mybir.EngineType.Pool: "Pool_",
    }

    band_nops = []       # per-quarter SP wait-carrier nops, filled at build
    act_nops = []        # per-ship ACT wait-carrier nops, filled at build
    prelude_nops = []    # (engine, nop) last-resort wait carriers, per context

    def _split_excess_waits(nc):
        """Walrus in this build caps sem waits per instruction (1 for DMA,
        2 otherwise).  Two legal rewrites bring Tile's output under the cap:
          - drop self-engine waits (WAW on a reused slot): engines complete
            in order, so an earlier same-engine producer is already done;
          - hoist remaining excess waits onto the nearest *preceding*
            same-engine instruction with spare capacity — the sequencer
            executes waits in program order, so waiting earlier is strictly
            more conservative.  (Producers of hoisted waits are tile-slot
            reuses >= one full quarter older, so no deadlock is possible.)
        """
        import bass_rust as _br

        nop_names = {i.ins.name for i in band_nops}
        act_nop_names = {i.ins.name for i in act_nops}
        prelude_by_name = {i.ins.name: i.ins for _, i in prelude_nops}

        # Re-seat the carrier nops: Tile's scheduler places them by its own
        # heuristics, but the deadlock safety rule needs each DMA's carriers
        # contiguous in front of it in the FINAL order.  Carriers are
        # interchangeable (all empty), so redistribute them and drop any
        # excess.  SP carriers precede SP loads; ACT carriers precede
        # ACT-issued output DMAs.
        for bb in nc.main_func.blocks:
            insts = list(bb.instructions)
            pool = [i for i in insts if i.name in nop_names]
            apool = [i for i in insts if i.name in act_nop_names]
            if not pool and not apool:
                continue
            rebuilt = []
            k = ak = 0
            for inst in insts:
                if inst.name in nop_names or inst.name in act_nop_names:
                    continue
                if (type(inst).__name__ == "InstDMACopy"
                        and inst.engine == mybir.EngineType.SP
                        and k < len(pool)):
                    take = pool[k:k + 3]
                    k += len(take)
                    rebuilt.extend(take)
                if (type(inst).__name__ == "InstDMACopy"
                        and inst.engine == mybir.EngineType.Activation
                        and ak < len(apool)):
                    take = apool[ak:ak + 3]
                    ak += len(take)
                    rebuilt.extend(take)
                rebuilt.append(inst)
            bb.instructions = rebuilt

        # Drop duplicate weight loads: the PE array retains loaded weights
        # across matmuls (HW-validated), so the second of two adjacent
        # identical InstLdweights is redundant.  Its waits duplicate the
        # first's and it carries no sem updates, so deletion is sem-safe.
        for bb in nc.main_func.blocks:
            rebuilt = []
            last_key = None
            for inst in bb.instructions:
                tn = type(inst).__name__
                if inst.engine == mybir.EngineType.PE:
                    if tn == "InstLdweights":
                        si = inst.sync_info
                        assert not (si and list(si.on_update))
                        key = (repr(inst.ins[0]), repr(inst.perf_mode),
                               repr(getattr(inst, "is_transpose", None)),
                               repr(inst.tile_position),
                               repr(inst.tile_size))
                        if key == last_key:
                            continue
                        last_key = key
                    elif tn != "InstMatmult":
                        last_key = None
                rebuilt.append(inst)
            bb.instructions = rebuilt

        def cap(inst):
            # Empirically this walrus accepts at most ONE sem wait per
            # instruction across every struct we hit (DMA, ACT, LW/matmul,
            # ctrl drain).
            return 1

        def set_waits(inst, waits):
            si = inst.sync_info
            ups = list(si.on_update) if si else []
            inst.sync_info = _br.SyncInfo(on_update=ups, on_wait=waits)

        def merge_wait(inst, w):
            """Add wait w to inst, merging same-sem waits by max value."""
            si = inst.sync_info
            waits = list(si.on_wait) if si else []
            for i, ex in enumerate(waits):
                if ex.ant_name == w.ant_name:
                    if w.wait_value > ex.wait_value:
                        waits[i] = w
                    set_waits(inst, waits)
                    return
            set_waits(inst, waits + [w])

        for bb in nc.main_func.blocks:
            cur_nops = []           # contiguous carrier-nop run on SP
            cur_act = []            # contiguous carrier-nop run on ACT
            streams = {}            # engine -> prior instructions, in order
            bb_preludes = {}        # engine -> prelude nops IN THIS BB only
            for inst in bb.instructions:
                stream = streams.setdefault(inst.engine, [])
                if inst.name in prelude_by_name:
                    bb_preludes.setdefault(inst.engine, []).append(inst)
                    stream.append(inst)
                    continue
                if inst.name in nop_names:
                    cur_nops.append(inst)
                    stream.append(inst)
                    continue
                if inst.name in act_nop_names:
                    cur_act.append(inst)
                    stream.append(inst)
                    continue
                # Only instructions that produce semaphore values (DMA
                # issues / explicit updates) invalidate their engine's
                # carrier run — helper RegisterMoves produce nothing a wait
                # could reference.
                is_sp_work = (
                    inst.engine == mybir.EngineType.SP
                    and (type(inst).__name__.startswith("InstDMA")
                         or bool(inst.sync_info
                                 and list(inst.sync_info.on_update))))
                is_act_work = (
                    inst.engine == mybir.EngineType.Activation
                    and (type(inst).__name__.startswith("InstDMA")
                         or bool(inst.sync_info
                                 and list(inst.sync_info.on_update))))
                si = inst.sync_info
                if si is None:
                    if is_sp_work:
                        cur_nops = []
                    if is_act_work:
                        cur_act = []
                    stream.append(inst)
                    continue
                waits = list(si.on_wait)
                if len(waits) <= cap(inst):
                    if is_sp_work:
                        cur_nops = []
                    if is_act_work:
                        cur_act = []
                    stream.append(inst)
                    continue
                # 1) drop self-engine waits (in-order engines: an earlier
                #    same-engine producer has completed by issue time)
                pfx = _ENG_SEM.get(inst.engine)
                waits = [w for w in waits
                         if not (pfx and w.ant_name.startswith(pfx))]
                # 1b) a WAR wait on the ACT dummy-read is implied by the WAR
                #     wait on the ACT-issued output DMA (same sequencer,
                #     in-order: dummy completed before the DMA was issued)
                if (len(waits) > cap(inst)
                        and any(w.ant_name.startswith("DMAHW") for w in waits)):
                    waits = [w for w in waits
                             if not w.ant_name.startswith("Activation_")]
                if len(waits) > cap(inst):
                    # keep one wait (prefer the DMA-lane RAW for DMAs), hoist
                    # the rest onto earlier same-engine instructions — waits
                    # execute in sequencer program order, so hoisting is
                    # strictly more conservative.  Producers of hoisted waits
                    # are tile-slot reuses from >= 2 pipeline stages earlier,
                    # so a bounded backward hoist cannot deadlock.
                    if type(inst).__name__ == "InstDMACopy":
                        keep = ([w for w in waits if w.ant_name.startswith("DMAHW")]
                                or waits)[:1]
                    else:
                        keep = waits[:1]
                    hoist = [w for w in waits if w not in keep]
                    for w in hoist:
                        placed = False
                        carrier_run = (
                            cur_nops if inst.engine == mybir.EngineType.SP
                            else cur_act
                            if inst.engine == mybir.EngineType.Activation
                            else None)
                        if carrier_run is not None:
                            # Carrier nops hold ONE sem each (walrus caps
                            # nop waits too).  Only the contiguous nop run
                            # immediately before this instruction is legal:
                            # every producer of this wait was issued before
                            # that run, so no self-deadlock is possible.
                            for cn in reversed(carrier_run):
                                csi = cn.sync_info
                                cw = list(csi.on_wait) if csi else []
                                if not cw:
                                    set_waits(cn, [w])
                                    placed = True
                                    break
                                if cw[0].ant_name == w.ant_name:
                                    merge_wait(cn, w)
                                    placed = True
                                    break
                        if not placed and inst.engine != mybir.EngineType.SP:
                            # (SP excluded: an SP instruction placed before
                            # this wait's producer would self-deadlock the
                            # sequencer; carriers above are the only safe
                            # spots.)
                            for prior in reversed(stream[-50:]):
                                psi = prior.sync_info
                                pw = list(psi.on_wait) if psi else []
                                if len(pw) < cap(prior):
                                    set_waits(prior, pw + [w])
                                    placed = True
                                    break
                        if not placed:
                            # last resort: prelude nop on this engine (they
                            # sit at the head of this context's stream)
                            for pn in bb_preludes.get(inst.engine, []):
                                psi = pn.sync_info
                                pw = list(psi.on_wait) if psi else []
                                same = [x for x in pw if x.ant_name == w.ant_name]
                                if same or len(pw) < 1:
                                    merge_wait(pn, w)
                                    placed = True
                                    break
                        assert placed, (
                            f"{inst.name}: no carrier for {w.ant_name}")
                    waits = keep
                assert len(waits) <= cap(inst), (
                    f"{inst.name}: still {len(waits)} waits")
                set_waits(inst, waits)
                if is_sp_work:
                    cur_nops = []
                if is_act_work:
                    cur_act = []
                stream.append(inst)

    dt = mybir.dt
    f32 = dt.float32
    bf16 = dt.bfloat16
    f8 = dt.float8e4
    DR = mybir.MatmulPerfMode.DoubleRowSwInterleave

    nc = bass.Bass(target_bir_lowering=False)
    # Few, large, fully-contiguous DMAs: HWDGE descriptor generation costs
    # ~625 ns per DMA instruction (shared across all engines), so the tensor
    # layouts are partition-major with everything a core streams per step in
    # one per-partition line.
    att8 = nc.dram_tensor("att8", [NQ // 2, 128, MC, 2, 2, QUART], f8,
                          kind="ExternalInput")
    # hf8 weights are host-pre-interleaved for DoubleRowSwInterleave:
    # hf8[p, mc, dch, 2k+i] = hf[m=mc*256+i*128+p, d=dch*128+127-k]
    # — the contiguous SBUF read keeps LDWEIGHTS on the fast (FWL-style)
    # path that plain DoubleRow's hardware interleave forfeits.  Partition-
    # major and identical to the SBUF tile layout, so the whole tensor
    # loads in ONE contiguous DMA.
    hf8 = nc.dram_tensor("hf8", [128, MC, DCH, 256], f8,
                         kind="ExternalInput")
    # rec[Q, pp, p, j, q] = rec_logical[d = (pp*2+j)*128 + p, Q*512 + q]
    rec = nc.dram_tensor("rec", [NQ, DCH // 2, 128, 2, QUART], bf16,
                         kind="ExternalOutput")

    with TileContext(nc) as tc:
        with (
            tc.tile_pool(name="hfp", bufs=2) as hfp,
            # 4 att slots: iteration r+1's pair loads overwrite iteration
            # r-1's slots, so the 8MB att stream overlaps a full iteration
            # of compute.  2 stage slots suffice (ship of d-pair k drains
            # while d-pair k+1 fills).
            tc.tile_pool(name="attp", bufs=4) as attp,
            tc.tile_pool(name="recp", bufs=2) as recp,
            tc.tile_pool(name="psp", bufs=1, space="PSUM") as psp,
        ):
            def _carried_load(dst, src, _n=[0]):
                # Two one-sem carrier nops immediately before each SP load
                # absorb its excess (WAR/WAW) waits; the load keeps one
                # DMA-lane wait.
                for i in (0, 1, 2):
                    band_nops.append(
                        nc.sync.nop(hint=f"carrier_{_n[0]}_{i}"))
                _n[0] += 1
                nc.sync.dma_start(dst, src)

            for _r in range(rep):
                for eng_name, eng in (("tensor", nc.tensor),
                                      ("vector", nc.vector),
                                      ("scalar", nc.scalar)):
                    for i in range(8):
                        prelude_nops.append(
                            (eng.engine,
                             eng.nop(hint=f"prelude_{eng_name}_{_r}_{i}")))

                # hf8 is SBUF-resident for the whole iteration; the
                # [128, 256] pre-interleaved weight slices come straight off
                # it.  Loaded inside the main context, 4-chunk groups
                # interleaved with quarter-0's att streams so the first
                # matmuls can start as soon as the first (hf, att) groups
                # land.
                hf_sb = hfp.tile([128, MC, DCH, 256], f8, tag="hf")

                if _r == 0:
                    # PE warm-up: a few matmuls on a zeroed scratch tile keep
                    # the PE clock ramping through the DMA fill so the first
                    # real matmuls run at full rate.  Six fit inside the
                    # ~3 us fill window even if their weight loads run fully
                    # serial on hardware, so they can never delay real work.
                    # They write psum slot p0, which quarter 0's d=0 chain
                    # then overwrites (start=True).
                    wsrc = hfp.tile([128, 2, QUART], f8, name="warm_src",
                                    tag="warm_src")
                    nc.vector.memset(wsrc[:, :, :], 0)
                    wps = psp.tile([128, QUART], f32, name="warm_ps",
                                   tag="p0")
                    for i in range(6):
                        nc.tensor.matmul(wps[:, :], wsrc[:, :, 0:128],
                                         wsrc[:, :, :], start=True,
                                         stop=True, perf_mode=DR)

                # Both att pairs (all four query quarters) are resident
                # before compute: each (mc, d) weight then serves FOUR
                # adjacent matmuls, so after the LDWEIGHTS dedup below only
                # 96 weight loads remain per iteration, each hidden under
                # the ~4x504-column matmul window (HW-validated: back-to-back
                # LDWEIGHTS cost ~160 ns serial; at >=4-matmul spacing they
                # pipeline for free).  The fill interleaves hf / pair-0 /
                # pair-1 chunk groups so the mc=0 matmuls start as soon as
                # the first groups land.
                at_p0 = attp.tile([128, MC, 2, 2, QUART], f8,
                                  name=f"at_p0_r{_r}", tag="at")
                at_p1 = attp.tile([128, MC, 2, 2, QUART], f8,
                                  name=f"at_p1_r{_r}", tag="at")
                # One whole-tile DMA per tensor (3 per iteration): big
                # contiguous loads sustain full HBM bandwidth (probe-
                # validated 327 GB/s), and a single completion semaphore
                # value per tile means the PE stream carries one satisfied
                # RAW wait per pass instead of one per 2-chunk group.
                _carried_load(hf_sb[:, :, :, :], hf8[:, :, :, :])
                _carried_load(at_p0[:, :, :, :, :], att8[0])
                _carried_load(at_p1[:, :, :, :, :], att8[1])
                pairs = (at_p0, at_p1)

                # Six single-d passes; each uses 4 psum banks (one per
                # quarter), alternating bank groups p0-3 / p4-7 so pass k+1
                # computes while pass k's DVE copies drain.  A d-pair shares
                # one stage tile per quarter, shipped as one ACT DMA into
                # the unchanged rec[Q, dp, p, j, q] layout.
                for dp in range(DCH // 2):
                    stages = {}
                    for Q in range(4):
                        stages[Q] = recp.tile(
                            [128, 2, QUART], bf16,
                            name=f"stage_{dp}_{Q}_r{_r}", tag=f"stage{Q}")
                    for j in (0, 1):
                        d = 2 * dp + j
                        grp = 4 * (d % 2)
                        pd = {}
                        for Q in range(4):
                            pd[Q] = psp.tile(
                                [128, QUART], f32,
                                name=f"ps_d{d}_{Q}_r{_r}",
                                tag=f"p{grp + Q}")
                        for mc in range(MC):
                            for Q in range(4):
                                nc.tensor.matmul(
                                    pd[Q][:, :],
                                    hf_sb[:, mc, d, :],
                                    pairs[Q // 2][:, mc, Q % 2, :, :],
                                    start=(mc == 0), stop=(mc == MC - 1),
                                    perf_mode=DR)
                        for Q in range(4):
                            # tiny psum read carries the PE wait so Tile
                            # elides it from the big copy (one-wait ISA cap)
                            tny = recp.tile([128, 1], f32,
                                            name=f"tny_d{d}_q{Q}_r{_r}",
                                            tag=f"tny{d}")
                            nc.vector.tensor_copy(tny[:], pd[Q][:, 0:1])
                            nc.vector.tensor_copy(stages[Q][:, j, :],
                                                  pd[Q][:, :])
                    for Q in range(4):
                        # ACT observes the DVE copies via this cheap read
                        dmy = recp.tile([128, 2, 1], bf16,
                                        name=f"dmy_{dp}_{Q}_r{_r}",
                                        tag="dmy")
                        nc.scalar.copy(dmy[:, :, :], stages[Q][:, :, 0:1])
                        # ACT carrier nops: re-seated directly before this
                        # ship's output DMA, they absorb its excess
                        # (cross-iteration WAR) waits so none can land on a
                        # head-of-block prelude before its producer.
                        for i in (0, 1, 2):
                            act_nops.append(nc.scalar.nop(
                                hint=f"actcarrier_{_r}_{dp}_{Q}_{i}"))
                        nc.scalar.dma_start(rec[Q, dp][:, 0:2, :],
                                            stages[Q][:, :, :])
    _split_excess_waits(nc)
    return nc


def _get_nc(rep: int = 1):
    if rep not in _NC:
        _NC[rep] = _build_nc(rep)
    return _NC[rep]


# ---------------------------------------------------------------- benchmark
BENCH_REP = 64                  # body iterations unrolled inside the bench NEFF


def bench(in_maps, iters: int = 10):
    """Steady-state per-iteration device time of the kernel body.

    NTFF profiling is unavailable under this axon client, and a single NEFF
    execution (~55 us) is two orders of magnitude below the per-dispatch
    axon RPC cost (~1 ms), so wall-timing single executions measures the
    tunnel, not the kernel.  Instead this benches a NEFF with the identical
    kernel body unrolled BENCH_REP times (each iteration re-streams every
    input from HBM and rewrites the full output), dispatches `iters` such
    executions asynchronously with the previous call's outputs donated as
    the next call's output buffers (no host<->device traffic in the timed
    loop), and reports wall / (iters * BENCH_REP): the steady-state device
    time of one kernel iteration, with dispatch overhead amortized.
    """
    import time

    import jax
    import numpy as np
    from jax.experimental.shard_map import shard_map
    from jax.sharding import Mesh, NamedSharding, PartitionSpec

    import concourse.bass2jax as bass2jax
    import concourse.mybir as mybir

    nc = _get_nc(BENCH_REP)
    bass2jax.install_neuronx_cc_hook()

    part_name = (nc.partition_id_tensor.name
                 if nc.partition_id_tensor is not None else None)
    in_names, out_names, out_avals, zero_outs = [], [], [], []
    for alloc in nc.m.functions[0].allocations:
        if not isinstance(alloc, mybir.MemoryLocationSet):
            continue
        name = alloc.memorylocations[0].name
        if alloc.kind == "ExternalInput":
            if name != part_name:
                in_names.append(name)
        elif alloc.kind == "ExternalOutput":
            shape = tuple(alloc.tensor_shape)
            dtype = mybir.dt.np(alloc.dtype)
            out_names.append(name)
            out_avals.append(jax.core.ShapedArray(shape, dtype))
            zero_outs.append(np.zeros(shape, dtype))
    n_params = len(in_names)
    n_outs = len(out_avals)
    all_names = in_names + out_names
    if part_name is not None:
        all_names = all_names + [part_name]
    donate = tuple(range(n_params, n_params + n_outs))

    def _body(*args):
        operands = list(args)
        if part_name is not None:
            operands.append(bass2jax.partition_id_tensor())
        outs = bass2jax._bass_exec_p.bind(
            *operands,
            out_avals=tuple(out_avals),
            in_names=tuple(all_names),
            out_names=tuple(out_names),
            lowering_input_output_aliases=(),
            sim_require_finite=True,
            sim_require_nnan=True,
            nc=nc,
        )
        return tuple(outs)

    devices = jax.devices()[:N_CORES]
    mesh = Mesh(np.asarray(devices), ("core",))
    sh = NamedSharding(mesh, PartitionSpec("core"))
    sharded = jax.jit(
        shard_map(_body, mesh=mesh,
                  in_specs=(PartitionSpec("core"),) * (n_params + n_outs),
                  out_specs=(PartitionSpec("core"),) * n_outs,
                  check_rep=False),
        donate_argnums=donate, keep_unused=True)

    concat_in = [
        np.concatenate([np.asarray(in_maps[c][nm]) for c in range(N_CORES)], 0)
        for nm in in_names
    ]
    dev_in = [jax.device_put(a, sh) for a in concat_in]
    mk_zeros = lambda: [
        jax.device_put(np.zeros((N_CORES * z.shape[0], *z.shape[1:]), z.dtype), sh)
        for z in zero_outs
    ]

    warm = sharded(*dev_in, *mk_zeros())
    jax.block_until_ready(warm)

    # Two-point slope estimator: the block_until_ready sync at the end of a
    # chain costs a fixed ~50-90 ms through the axon tunnel regardless of
    # work, so wall times of a short chain (K1 calls) and a long chain (K2
    # calls) are each min'd over several groups (min is the noise-robust
    # estimator of the no-interference wall) and the slope
    # (t_K2 - t_K1) / (K2 - K1) cancels the fixed sync cost exactly,
    # leaving the steady-state device time of one chained call.
    # Sustained PE load self-throttles the part within a fraction of a
    # second and recovers after ~3 s idle (measured: chain slope degrades
    # from ~5.0 ms/call over calls 2..8 to ~6.6 ms/call over calls 32..64,
    # fully repeatable), and every sync round-trip costs a fixed ~95 ms
    # through the axon tunnel.  So: each round sleeps 3 s to cool, then
    # times two SHORT chains (both well under the heating timescale); the
    # slope of the min walls cancels the fixed sync cost and reports the
    # cool-state per-iteration device time.  Keep whatever completed if
    # the device wedges mid-bench (NRT_EXEC_UNIT_UNRECOVERABLE happens).
    # The fixed sync cost varies +-tens of ms BETWEEN syncs, so slopes must
    # be paired within a round (same system state) and Delta must be large
    # enough to drown per-sync variance; the median over rounds is robust
    # to the remaining two-sided noise without the min's downward bias.
    # K2*BENCH_REP iterations ~0.25 s of work keeps self-heating modest.
    K1 = 2
    K2 = 32
    rounds = 12
    slopes = []
    cur = warm
    try:
        # one deeper cool-down first: the compile/correctness work that ran
        # just before bench leaves a warm state that 3 s does not fully
        # clear (observed: a run where all 12 rounds sat 20% above the
        # cool-state cluster)
        time.sleep(12.0)
        for _ in range(rounds):
            time.sleep(6.0)
            walls = []
            for K in (K1, K2):
                t0 = time.perf_counter()
                for i in range(K):
                    cur = sharded(*dev_in, *cur)
                jax.block_until_ready(cur)
                walls.append(time.perf_counter() - t0)
            slopes.append((walls[1] - walls[0]) / ((K2 - K1) * BENCH_REP))
    except Exception:
        if not slopes:
            raise
    slopes.sort()
    print("bench slopes (us/iter): "
          + " ".join(f"{s*1e6:.1f}" for s in slopes))
    slope_ns = slopes[len(slopes) // 2] * 1e9
    if slope_ns <= 0:
        slope_ns = min(s for s in slopes if s > 0) * 1e9
    return slope_ns, cur


# ------------------------------------------------------------------- kernel
def _prepare(x_hr, x_lr_inpainted, attn_map, x_lr_blurred):
    """Host sharding prep: upsample, unfold, fp8 quantize, per-core tiles."""
    import ml_dtypes

    npf8 = ml_dtypes.float8_e4m3

    x_hr = np.asarray(x_hr, np.float32)
    x_lr_inpainted = np.asarray(x_lr_inpainted, np.float32)
    attn_map = np.asarray(attn_map, np.float32)
    x_lr_blurred = np.asarray(x_lr_blurred, np.float32)

    blur_hr = _upsample2(x_lr_blurred)                    # (B, C, 512, 512)
    base = _upsample2(x_lr_inpainted)                     # (B, C, 512, 512)

    q_starts = (0, L - LQ)                                # 0 and 1953
    in_maps = []
    hf8_cache = {}
    att8_cache = {}
    for core in range(N_CORES):
        b, half = core // 2, core % 2
        if b not in hf8_cache:
            hfp = np.zeros((MP, CKK), npf8)
            hfp[:L] = _unfold_hf(x_hr[b], blur_hr[b]).astype(npf8)
            # SwInterleave weight layout, partition-major (one-DMA load):
            # [p, mc, dch, 2k+i] = hfp[mc*256 + i*128 + p, dch*128 + 127 - k]
            h6 = hfp.reshape(MC, 2, 128, DCH, 128)[..., ::-1]
            hil = np.ascontiguousarray(
                h6.transpose(0, 2, 3, 4, 1)).reshape(MC, 128, DCH, 256)
            hf8_cache[b] = np.ascontiguousarray(hil.transpose(1, 0, 2, 3))
            att8_cache[b] = (attn_map[b, 0] * SCALE).astype(npf8)  # (L, L)
        q0 = q_starts[half]
        ap = np.zeros((LQP, MP), npf8)
        ap[:LQ, :L] = att8_cache[b][q0:q0 + LQ, :]
        # [Q*504 + q, mc*256 + i*128 + p] -> [pair, p, mc, Qi, i, q]
        at = np.ascontiguousarray(
            ap.reshape(NQ // 2, 2, QUART, MC, 2, 128)
            .transpose(0, 5, 3, 1, 4, 2))
        in_maps.append({"att8": at, "hf8": hf8_cache[b]})
    return in_maps, base


def _finish(per_core_rec, base):
    """Gather: stitch q-halves, descale, fold, normalize, add base."""
    cols = np.empty((B, CKK, L), np.float32)

    def _unpack(r):
        # rec tensor is [Q, pp, p, j, q] -> logical [(pp*2+j)*128+p, Q*512+q]
        return np.asarray(r).transpose(1, 3, 2, 0, 4) \
            .reshape(CKK, LQP).astype(np.float32)

    for b in range(B):
        rec_a = _unpack(per_core_rec[2 * b])
        rec_b = _unpack(per_core_rec[2 * b + 1])
        cols[b, :, :LQ] = rec_a[:, :LQ]
        cols[b, :, LQ:] = rec_b[:, 2 * LQ - L:LQ]
    img = _fold(cols)
    out = base + img / (_norm_map() * SCALE)
    return out.astype(np.float32)


def kernel(x_hr, x_lr_inpainted, attn_map, x_lr_blurred):
    global LAST_RESULT
    from concourse.bass_utils import run_bass_kernel_spmd

    in_maps, base = _prepare(x_hr, x_lr_inpainted, attn_map, x_lr_blurred)
    nc = _get_nc()
    trace = bool(os.environ.get("KERNEL_TRACE"))
    res = run_bass_kernel_spmd(nc, in_maps, list(range(N_CORES)), trace=trace)
    LAST_RESULT = res
    return _finish([res.results[c]["rec"] for c in range(N_CORES)], base)

